# revision 3
# baseline (speedup 1.0000x reference)
"""Trainium2 Bass kernel for an LSTM + per-step Linear head.

Model (PyTorch gate order i,f,g,o):
    gates_t = x_t @ W_ih.T + h_t @ W_hh.T + (b_ih + b_hh)      [m, 2048]
    c_{t+1} = sig(f)*c_t + sig(i)*tanh(g)
    h_{t+1} = sig(o)*tanh(c_{t+1})
    out_t   = h_{t+1} @ W_out.T + b_out                         [m, 256]
Output: [TX, M, 256] stacked over t.

Sharding: data-parallel over batch m=4096 across 8 cores (512 rows each);
weights replicated. On-chip layout is gate-major ("transposed"): activations
h,c live as [feature, m] so the feature dim sits on SBUF partitions and is
the matmul contraction dim. x_t arrives via a transposing (xbar) DMA straight
from DRAM (X is pre-cast to fp16 on the host, which is lossless w.r.t. the
fp16 matmuls that consume it). The output projection flips back to [m, n]
naturally by using h^T as the stationary operand. All matmul operands are
fp16 (1 cycle/row on the PE, fp32 PSUM accumulate); the cell state c stays
fp32 on the DVE. The kernel is PE-bound at ~98% tensor-engine occupancy:
gates = 96 MMs x 512 cols + out-proj 16 MMs x 256 cols per step.
"""

import sys

sys.path.insert(0, "/opt/trn_rl_repo")

import numpy as np

M, TX, NV, NA = 4096, 128, 256, 512
NG = 4 * NA  # 2048 gate rows
N_CORES = 8
M_LOC = M // N_CORES  # 512
MC = M_LOC // 128  # 4 m-chunks
GC = NG // 128  # 16 gate chunks
KX = NV // 128  # 2 contraction chunks for the x part
KH = NA // 128  # 4 contraction chunks for the h part

_CACHE = {}


def _build(tx: int):
    import concourse.bass as bass
    import concourse.mybir as mybir
    import concourse.tile as tile
    from concourse import bacc

    f32 = mybir.dt.float32
    f16 = mybir.dt.float16
    ACT_SIG = mybir.ActivationFunctionType.Sigmoid
    ACT_TANH = mybir.ActivationFunctionType.Tanh

    nc = bacc.Bacc("TRN2", target_bir_lowering=False, debug=False,
                   num_devices=N_CORES)

    X_d = nc.declare_dram_parameter("X", [M_LOC, tx, NV], f16, isOutput=False)
    h0_d = nc.declare_dram_parameter("h0T", [NA, M_LOC], f16, isOutput=False)
    c0_d = nc.declare_dram_parameter("c0T", [NA, M_LOC], f32, isOutput=False)
    wih_d = nc.declare_dram_parameter("WihT", [NV, NG], f16, isOutput=False)
    whh_d = nc.declare_dram_parameter("WhhT", [NA, NG], f16, isOutput=False)
    wout_d = nc.declare_dram_parameter("WoutT", [NA, NV], f16, isOutput=False)
    bias_d = nc.declare_dram_parameter("bias", [NG, 1], f32, isOutput=False)
    bout_d = nc.declare_dram_parameter("bout", [128, NV], f32, isOutput=False)
    Y_d = nc.declare_dram_parameter("Y", [tx, M_LOC, NV], f32, isOutput=True)

    with tile.TileContext(nc) as tc:
        from contextlib import ExitStack

        with ExitStack() as ctx:
            wpool = ctx.enter_context(tc.tile_pool(name="w", bufs=1))
            hpool = ctx.enter_context(tc.tile_pool(name="h", bufs=2))
            cpool = ctx.enter_context(tc.tile_pool(name="c", bufs=2))
            xtpool = ctx.enter_context(tc.tile_pool(name="xt", bufs=3))
            apool = ctx.enter_context(tc.tile_pool(name="a", bufs=2))
            tpool = ctx.enter_context(tc.tile_pool(name="t", bufs=4))
            opool = ctx.enter_context(tc.tile_pool(name="o", bufs=3))
            ps_g = ctx.enter_context(tc.tile_pool(name="psg", bufs=6, space="PSUM"))
            ps_o = ctx.enter_context(tc.tile_pool(name="pso", bufs=2, space="PSUM"))

            # ---- constants / weights (one-time loads) ----
            wih = []
            for kc in range(KX):
                w = wpool.tile([128, NG], f16, tag=f"wih{kc}")
                nc.sync.dma_start(w[:], wih_d[kc * 128:(kc + 1) * 128, :])
                wih.append(w)
            whh = []
            for kc in range(KH):
                w = wpool.tile([128, NG], f16, tag=f"whh{kc}")
                nc.sync.dma_start(w[:], whh_d[kc * 128:(kc + 1) * 128, :])
                whh.append(w)
            wout = []
            for kc in range(KH):
                w = wpool.tile([128, NV], f16, tag=f"wout{kc}")
                nc.sync.dma_start(w[:], wout_d[kc * 128:(kc + 1) * 128, :])
                wout.append(w)
            bias_t = []
            for gc in range(GC):
                b = wpool.tile([128, 1], f32, tag=f"b{gc}")
                nc.sync.dma_start(b[:], bias_d[gc * 128:(gc + 1) * 128, :])
                bias_t.append(b)
            bout_sb = wpool.tile([128, NV], f32, tag="bout")
            nc.sync.dma_start(bout_sb[:], bout_d[:])

            # ---- initial state ----
            h_cur, c_cur = [], []
            for kc in range(KH):
                h = hpool.tile([128, M_LOC], f16, tag=f"h{kc}")
                nc.sync.dma_start(h[:], h0_d[kc * 128:(kc + 1) * 128, :])
                h_cur.append(h)
                c = cpool.tile([128, M_LOC], f32, tag=f"c{kc}")
                nc.sync.dma_start(c[:], c0_d[kc * 128:(kc + 1) * 128, :])
                c_cur.append(c)

            def x_prefetch(t):
                """Transposing DMA: X[:, t, fc] DRAM [512m,128f] -> SBUF [128f,512m]."""
                xt = []
                for fc in range(KX):
                    sb = xtpool.tile([128, M_LOC], f16, tag=f"xt{fc}")
                    nc.sync.dma_start_transpose(
                        sb[:], X_d[:, t, fc * 128:(fc + 1) * 128])
                    xt.append(sb)
                return xt

            xt_cur = x_prefetch(0)

            for t in range(tx):
                xt_next = x_prefetch(t + 1) if t + 1 < tx else None

                # gates (gate-major): psum[gc] = Wih.T[:,gc].T @ xT + Whh.T[:,gc].T @ hT
                acts = []
                for gc in range(GC):
                    sl = slice(gc * 128, (gc + 1) * 128)
                    ps = ps_g.tile([128, M_LOC], f32, tag="psg")
                    for kc in range(KX):
                        nc.tensor.matmul(ps[:], wih[kc][:, sl], xt_cur[kc][:],
                                         start=(kc == 0), stop=False)
                    for kc in range(KH):
                        nc.tensor.matmul(ps[:], whh[kc][:, sl], h_cur[kc][:],
                                         start=False, stop=(kc == KH - 1))
                    a = apool.tile([128, M_LOC], f32, tag=f"a{gc}")
                    func = ACT_TANH if 8 <= gc < 12 else ACT_SIG
                    nc.scalar.activation(a[:], ps[:], func, bias=bias_t[gc][:])
                    acts.append(a)

                # state update per feature chunk: c' = f*c + i*g~ ; h' = o*tanh(c')
                h_new, c_new = [], []
                for cc in range(KH):
                    i_s, f_s, g_t, o_s = (acts[cc], acts[4 + cc], acts[8 + cc],
                                          acts[12 + cc])
                    cn = cpool.tile([128, M_LOC], f32, tag=f"c{cc}")
                    nc.vector.tensor_mul(cn[:], f_s[:], c_cur[cc][:])
                    tm = tpool.tile([128, M_LOC], f32, tag="tmp")
                    nc.vector.tensor_mul(tm[:], i_s[:], g_t[:])
                    nc.vector.tensor_add(cn[:], cn[:], tm[:])
                    tc_t = tpool.tile([128, M_LOC], f32, tag="tanhc")
                    nc.scalar.activation(tc_t[:], cn[:], ACT_TANH)
                    hn = hpool.tile([128, M_LOC], f16, tag=f"h{cc}")
                    nc.vector.tensor_mul(hn[:], o_s[:], tc_t[:])
                    c_new.append(cn)
                    h_new.append(hn)

                # out_t[m, nv] = h'(t)^T.T @ WoutT + 1.T @ bout
                for mc in range(MC):
                    msl = slice(mc * 128, (mc + 1) * 128)
                    po = ps_o.tile([128, NV], f32, tag="pso")
                    for kc in range(KH):
                        nc.tensor.matmul(po[:], h_new[kc][:, msl], wout[kc][:],
                                         start=(kc == 0), stop=(kc == KH - 1))
                    ob = opool.tile([128, NV], f32, tag=f"ob{mc}")
                    nc.vector.tensor_add(ob[:], po[:], bout_sb[:])
                    nc.sync.dma_start(Y_d[t, msl, :], ob[:])

                h_cur, c_cur = h_new, c_new
                xt_cur = xt_next

    nc.compile()
    return nc


def _get_nc(tx: int):
    if tx not in _CACHE:
        _CACHE[tx] = _build(tx)
    return _CACHE[tx]


def kernel(X, a0, c0, W_ih, W_hh, b_ih, b_hh, W_out, b_out):
    import os, time
    from concourse.bass_utils import run_bass_kernel_spmd

    timing = os.environ.get("BASS_KERNEL_TIMING")
    t0 = time.time()

    tx = X.shape[1]
    nc = _get_nc(tx)
    if timing:
        print(f"[timing] build+compile(cached?): {time.time()-t0:.3f}s", flush=True)
    t0 = time.time()

    f32 = np.float32
    f16 = np.float16
    wihT = np.ascontiguousarray(np.asarray(W_ih, f32).T.astype(f16))
    whhT = np.ascontiguousarray(np.asarray(W_hh, f32).T.astype(f16))
    woutT = np.ascontiguousarray(np.asarray(W_out, f32).T.astype(f16))
    bias = np.ascontiguousarray(
        (np.asarray(b_ih, f32) + np.asarray(b_hh, f32)).reshape(NG, 1))
    bout = np.ascontiguousarray(
        np.broadcast_to(np.asarray(b_out, f32).reshape(1, NV), (128, NV)))
    a0T = np.ascontiguousarray(np.asarray(a0, f32).T.astype(f16))
    c0T = np.ascontiguousarray(np.asarray(c0, f32).T)
    X = np.ascontiguousarray(np.asarray(X, f32).astype(f16))

    in_maps = []
    for c in range(N_CORES):
        sl = slice(c * M_LOC, (c + 1) * M_LOC)
        in_maps.append({
            "X": X[sl],
            "h0T": np.ascontiguousarray(a0T[:, sl]),
            "c0T": np.ascontiguousarray(c0T[:, sl]),
            "WihT": wihT, "WhhT": whhT, "WoutT": woutT,
            "bias": bias, "bout": bout,
        })

    if timing:
        print(f"[timing] host prep: {time.time()-t0:.3f}s", flush=True)
    t0 = time.time()

    global _LAST_RES
    res = run_bass_kernel_spmd(nc, in_maps, core_ids=list(range(N_CORES)),
                               trace=TRACE)
    _LAST_RES = res
    if timing:
        print(f"[timing] spmd run: {time.time()-t0:.3f}s", flush=True)
    t0 = time.time()
    out = np.empty((tx, M, NV), f32)
    for c in range(N_CORES):
        out[:, c * M_LOC:(c + 1) * M_LOC, :] = res.results[c]["Y"]
    if timing:
        print(f"[timing] gather: {time.time()-t0:.3f}s", flush=True)
    return out


TRACE = False
_LAST_RES = None

